# revision 20
# baseline (speedup 1.0000x reference)
"""Trainium2 Bass kernel for nn_MimicLoss (masked-MSE mimic loss), v9.

Data-parallel over batch: 8 NeuronCores x 4 samples. Per core:
  1. rasterize per-sample union-of-positive-boxes masks: priors arrive
     host-transposed so cx/cy/w/h are contiguous [128, 64] rows;
     coverage built in 6 batched [128, 4096] bf16 DVE ops, xb4
     (cov_x * pos) batched 16 chunks per DVE op, then one bf16 matmul
     per 128-prior chunk accumulates
       hit[h, (b,w)] += sum_p cov_y[p,h] * (cov_x[p,w] * pos[p,b])
     mask = hit > 0; mask also transposed to per-sample row layout
     mask_rows[b, (h w)] via 4 small SBUF->SBUF DMAs.
  2. stream units u = (pixel-half h, sample b): tile [128, cc, j] f32
     holds s[b, :, h*2048:(h+1)*2048]; s and t interleave on the sync
     HWDGE queue (one queue sustains ~385 GB/s). d = s - t in place in
     the s tile; subtracts are deadline-scheduled: DVE interleaves them
     between raster ops so each unit's subtract lands before its t/s
     buffer must recycle, with units 2 and 4 offloaded to GpSimd (kept
     few: GpSimd tensor ops degrade concurrent DVE throughput). ACT
     squares to f32r; PE accumulates per-sample column sums into ONE
     psum tile [4, 2048] per pixel-half using per-sample indicator
     stationaries sel_b [128, 4] (only column b set), so sample b's
     colsum accumulates in psum row b across the phase.
  3. per pixel-half: DVE multiplies the psum rows directly with
     mask_rows into scr, one reduce per region -> ccol columns; msum
     via one reduce over mask_rows. Host combines the [4, 4] outputs:
     contrib = cols 0..2, msum = col 3; applies /(msum*C) and the
     empty-mask-resets-loss scan, /B.

Self-contained: shapes hardcoded for map_t2/map_s2 [32,256,64,64] f32,
priors [8192,4] f32, mimic_label [32,8192] int32.
"""
import sys

sys.path.insert(0, "/opt/trn_rl_repo")

import numpy as np

import concourse.bacc as bacc
import concourse.tile as tile
from concourse import mybir
from concourse.alu_op_type import AluOpType as Op

F32 = mybir.dt.float32
F32R = mybir.dt.float32r
I32 = mybir.dt.int32
BF16 = mybir.dt.bfloat16
AF = mybir.ActivationFunctionType

B, C, H, W = 32, 256, 64, 64
P = 8192
N_CORES = 8
BPC = B // N_CORES          # samples per core
HW = H * W                  # 4096
HWH = HW // 2               # 2048 pixels per half
HWQ = HWH // 2              # 1024 pixels per tail part
NCHUNK = 64                 # prior chunks; prior p = lane*64 + c
CC = C // 128               # channel chunks
GCH = 16                    # chunks per xb4 batch group
NGR = NCHUNK // GCH         # 4 raster groups
NU = 2 * BPC                # stream units (pixel-half, sample)
GP_SUB_UNITS = ()           # GpSimd tensor ops degrade concurrent DVE ops


def build_nc():
    nc = bacc.Bacc("TRN2", debug=False)

    s = nc.dram_tensor("s", [BPC, C, H, W], F32, kind="ExternalInput")
    t = nc.dram_tensor("t", [BPC, C, H, W], F32, kind="ExternalInput")
    # host-transposed, contiguous per partition:
    # priors_r[l, j*64 + c] = priors[l*64 + c, j]
    priors_r = nc.dram_tensor("priors_r", [128, 4 * NCHUNK], F32, kind="ExternalInput")
    # labels_r[l, c*BPC + b] = mimic_label[b, l*64 + c] (host-permuted)
    labels_r = nc.dram_tensor("labels_r", [128, NCHUNK * BPC], I32, kind="ExternalInput")
    # sel[l, b, :] = indicator row: only column b set (host-provided f32r)
    sel_in = nc.dram_tensor("sel", [128, BPC, BPC], F32R, kind="ExternalInput")
    out = nc.dram_tensor("out", [BPC, 4], F32, kind="ExternalOutput")
    out2 = nc.dram_tensor("out2", [1, BPC], F32, kind="ExternalOutput")

    # unit (b, h) tile layout: [p, cc, j] = x[b, cc*128 + p, h*2048 + j]
    s_r = s[:].rearrange("b (cc p) (h r) w -> b h p cc (r w)", cc=CC, h=2)
    t_r = t[:].rearrange("b (cc p) (h r) w -> b h p cc (r w)", cc=CC, h=2)

    with tile.TileContext(nc) as tc:
        with (
            tc.tile_pool(name="small", bufs=1) as small,
            tc.tile_pool(name="xb4p", bufs=1) as xb4p,
            tc.tile_pool(name="stream_s", bufs=3) as pool_s,
            tc.tile_pool(name="stream_t", bufs=3) as pool_t,
            tc.tile_pool(name="d2p", bufs=2) as d2p,
            tc.tile_pool(name="psum", bufs=1, space="PSUM") as psump,
        ):
            constp = small
            rastp = small
            ps_hit = psump
            ps_cs = psump
            ps_ms = psump
            # ---- constants & small inputs ----
            # sel3d[:, b, :] = stationary for sample b (only column b set)
            sel_r = constp.tile([128, BPC, BPC], F32R)
            nc.scalar.dma_start(sel_r[:], sel_in[:])

            priors_sb = small.tile([128, 4, NCHUNK], F32)
            nc.scalar.dma_start(
                priors_sb[:].rearrange("l j c -> l (j c)"), priors_r[:]
            )
            labels_sb = small.tile([128, NCHUNK, BPC], I32)
            nc.scalar.dma_start(
                labels_sb[:], labels_r[:].rearrange("l (c b) -> l c b", b=BPC)
            )

            # ---- stream tiles + DMA emission (sync HWDGE queue) ----
            # unit u: h = u // BPC, b = u % BPC. Last unit split into
            # column-halves (parts) for a progressive tail drain.
            def unit_bh(u):
                h, b = divmod(u, BPC)
                return b, h

            # s on the sync HWDGE ring; t on the scalar HWDGE ring.
            # t3.. are issued later (interleaved after squares) so the
            # t-buffer rotation waits never block ACT's sequencer.
            TAIL_PARTS = ((0, 1024), (1024, 512), (1536, 512))
            unit_parts = [None] * NU  # list of (s_tile, t_tile, col0, ncols)
            for u in range(NU):
                b, h = unit_bh(u)
                s_t = pool_s.tile([128, CC, HWH], F32, tag="s", name=f"s_{u}")
                t_t = pool_t.tile([128, CC, HWH], F32, tag="t", name=f"t_{u}")
                if u < NU - 1:
                    nc.sync.dma_start(s_t[:], s_r[b, h])
                    unit_parts[u] = [(s_t, t_t, 0, HWH)]
                else:
                    parts = []
                    for col0, ncols in TAIL_PARTS:
                        cols = slice(col0, col0 + ncols)
                        nc.scalar.dma_start(s_t[:, :, cols], s_r[b, h][:, :, cols])
                        parts.append((s_t, t_t, col0, ncols))
                    unit_parts[u] = parts

            def emit_t_load(u):
                b, h = unit_bh(u)
                for s_t, t_t, col0, ncols in unit_parts[u]:
                    cols = slice(col0, col0 + ncols)
                    nc.scalar.dma_start(t_t[:, :, cols], t_r[b, h][:, :, cols])

            for u in range(3):
                emit_t_load(u)

            # ---- raster tiles ----
            iota_rep = rastp.tile([128, NCHUNK, 64], BF16)
            covx_all = rastp.tile([128, NCHUNK, 64], BF16)
            covy_all = rastp.tile([128, NCHUNK, 64], BF16)
            tmpB = rastp.tile([128, NCHUNK, 64], BF16)
            pos_f = small.tile([128, NCHUNK * BPC], F32)
            pos_bf = small.tile([128, NCHUNK, BPC], BF16)
            hw_half = small.tile([128, NCHUNK], F32)
            hh_half = small.tile([128, NCHUNK], F32)
            xm1 = small.tile([128, NCHUNK], F32)
            xx1 = small.tile([128, NCHUNK], F32)
            ym1 = small.tile([128, NCHUNK], F32)
            yy1 = small.tile([128, NCHUNK], F32)

            hit = ps_hit.tile([64, BPC * 64], F32)
            mask_f = small.tile([64, BPC * 64], F32)
            mask_rows = small.tile([BPC, HW], F32)
            cs0_sb = small.tile([BPC, HWH], F32)
            ccol = small.tile([BPC, 4], F32)
            ones64 = constp.tile([64, 1], F32)
            ms_ps = ps_ms.tile([1, BPC * 64], F32)
            ms_sb = small.tile([1, BPC], F32)

            def bcast(bnd):
                return bnd[:].rearrange("p (c o) -> p c o", o=1).broadcast_to(
                    [128, NCHUNK, 64]
                )

            # ---- gpsimd program start: iota ----
            nc.gpsimd.iota(
                iota_rep[:],
                pattern=[[0, NCHUNK], [1, 64]],
                base=0,
                channel_multiplier=0,
                allow_small_or_imprecise_dtypes=True,
            )

            # ---- engine-op emitters (called in deadline order below) ----
            def emit_prep():
                nc.vector.memset(ones64[:], 1.0)
                nc.vector.tensor_copy(
                    pos_f[:], labels_sb[:].rearrange("p c b -> p (c b)")
                )
                nc.vector.tensor_single_scalar(pos_f[:], pos_f[:], 0.0, Op.is_gt)
                nc.vector.tensor_copy(
                    pos_bf[:].rearrange("p c b -> p (c b)"), pos_f[:]
                )
                cx = priors_sb[:, 0, :]
                cy = priors_sb[:, 1, :]
                bw = priors_sb[:, 2, :]
                bh = priors_sb[:, 3, :]
                nc.vector.tensor_single_scalar(hw_half[:], bw, 0.5, Op.mult)
                nc.vector.tensor_single_scalar(hh_half[:], bh, 0.5, Op.mult)
                nc.vector.tensor_tensor(xm1[:], cx, hw_half[:], Op.subtract)
                nc.vector.tensor_scalar(xm1[:], xm1[:], 64.0, -1.0, Op.mult, Op.add)
                nc.vector.tensor_tensor(xx1[:], cx, hw_half[:], Op.add)
                nc.vector.tensor_scalar(xx1[:], xx1[:], 64.0, -1.0, Op.mult, Op.add)
                nc.vector.tensor_tensor(ym1[:], cy, hh_half[:], Op.subtract)
                nc.vector.tensor_scalar(ym1[:], ym1[:], 64.0, -1.0, Op.mult, Op.add)
                nc.vector.tensor_tensor(yy1[:], cy, hh_half[:], Op.add)
                nc.vector.tensor_scalar(yy1[:], yy1[:], 64.0, -1.0, Op.mult, Op.add)

            cs_ps = [None, None]

            def emit_sub(u, part=None):
                b, h = unit_bh(u)
                parts = unit_parts[u] if part is None else [unit_parts[u][part]]
                for s_t, t_t, col0, ncols in parts:
                    cols = slice(col0, col0 + ncols)
                    eng = nc.gpsimd if u in GP_SUB_UNITS else nc.vector
                    eng.tensor_tensor(
                        s_t[:, :, cols], s_t[:, :, cols], t_t[:, :, cols],
                        Op.subtract,
                    )

            def emit_sq_mm(u, part=None):
                b, h = unit_bh(u)
                if cs_ps[h] is None or (b == 0 and part in (None, 0)):
                    if cs_ps[h] is None:
                        cs_ps[h] = ps_cs.tile(
                            [BPC, HWH], F32, tag="cs", name=f"cs{h}"
                        )
                first = b == 0
                last = b == BPC - 1
                parts = unit_parts[u] if part is None else [unit_parts[u][part]]
                for s_t, t_t, col0, ncols in parts:
                    cols = slice(col0, col0 + ncols)
                    d2 = d2p.tile([128, CC, HWH], F32R, tag="d2", name="d2")
                    nc.scalar.activation(d2[:, :, cols], s_t[:, :, cols], AF.Square)
                    if part in (None, 0) and u + 3 < NU:
                        emit_t_load(u + 3)
                    nq = ncols // 512
                    for cc in range(CC):
                        for q in range(nq):
                            c0 = col0 + q * 512
                            nc.tensor.matmul(
                                cs_ps[h][0:BPC, c0 : c0 + 512],
                                sel_r[:, b, :],
                                d2[:, cc, c0 : c0 + 512],
                                start=(first and cc == 0),
                                stop=(last and cc == CC - 1),
                            )

            def emit_xb4_mms(g):
                xb4 = xb4p.tile([128, GCH, BPC, 64], BF16, tag="xb4", name="xb4")
                covx_v = (
                    covx_all[:, g * GCH : (g + 1) * GCH, :]
                    .rearrange("p c (o w) -> p c o w", o=1)
                    .broadcast_to([128, GCH, BPC, 64])
                )
                pos_v = (
                    pos_bf[:, g * GCH : (g + 1) * GCH, :]
                    .rearrange("p c (b o) -> p c b o", o=1)
                    .broadcast_to([128, GCH, BPC, 64])
                )
                nc.vector.tensor_tensor(xb4[:], covx_v, pos_v, Op.mult)
                for k in range(GCH):
                    c = g * GCH + k
                    nc.tensor.matmul(
                        hit[:],
                        covy_all[:, c, :],
                        xb4[:, k].rearrange("p b w -> p (b w)"),
                        start=(c == 0),
                        stop=(c == NCHUNK - 1),
                    )

            def emit_mask_rows():
                # 4 SBUF->SBUF DMAs: [64, 64] block -> one [1, 4096] row
                for b in range(BPC):
                    nc.gpsimd.dma_start(
                        mask_rows[b : b + 1, :].rearrange("o (p j) -> o p j", j=64),
                        mask_f[:, b * 64 : (b + 1) * 64],
                    )

            def emit_dot(src_ap, h, col0, ncols, ccol_idx):
                # in-place: mask_rows *= cs rows, then reduce the region
                cols = slice(col0, col0 + ncols)
                mcols = slice(h * HWH + col0, h * HWH + col0 + ncols)
                nc.vector.tensor_tensor(
                    mask_rows[:, mcols], src_ap[:, cols], mask_rows[:, mcols],
                    Op.mult,
                )
                nc.vector.tensor_reduce(
                    ccol[:, ccol_idx : ccol_idx + 1],
                    mask_rows[:, mcols],
                    mybir.AxisListType.X,
                    Op.add,
                )

            # ---- explicit deadline-ordered schedule ----
            # DVE: prep -> sub0 -> 2 coverage ops -> sub1 -> 4 coverage
            # ops -> xb4 g0 -> sub3 -> xb4 g1,g2 -> sub5 -> xb4 g3 ->
            # mask -> sub6 -> msum -> dot_h0 -> sub7a,b -> dots h1.
            # GpSimd: iota -> sub2 -> sub4 -> mask_rows.
            emit_prep()
            emit_sub(0)
            emit_sq_mm(0)
            nc.vector.tensor_tensor(covy_all[:], iota_rep[:], bcast(ym1), Op.is_gt)
            nc.vector.tensor_tensor(covx_all[:], iota_rep[:], bcast(xm1), Op.is_gt)
            emit_sub(1)
            emit_sq_mm(1)
            nc.vector.tensor_tensor(tmpB[:], iota_rep[:], bcast(xx1), Op.is_le)
            nc.vector.tensor_tensor(covx_all[:], covx_all[:], tmpB[:], Op.mult)
            nc.vector.tensor_tensor(tmpB[:], iota_rep[:], bcast(yy1), Op.is_le)
            nc.vector.tensor_tensor(covy_all[:], covy_all[:], tmpB[:], Op.mult)
            emit_sub(2)       # GpSimd
            emit_sq_mm(2)
            emit_xb4_mms(0)
            emit_sub(3)
            emit_sq_mm(3)
            # h0 psum closed: one cheap ACT copy frees the psum buffer
            # so PE's h1 start-matmuls don't wait on the late h0 dot
            nc.scalar.copy(cs0_sb[:], cs_ps[0][:])
            emit_xb4_mms(1)
            emit_sub(4)       # GpSimd
            emit_sq_mm(4)
            emit_xb4_mms(2)
            emit_sub(5)
            emit_sq_mm(5)
            emit_sub(6)
            emit_sq_mm(6)
            emit_xb4_mms(3)
            nc.vector.tensor_single_scalar(mask_f[:], hit[:], 0.0, Op.is_gt)
            emit_mask_rows()  # GpSimd
            # msum: PE column-sum of mask_f, then 4 tiny per-sample reduces
            nc.tensor.matmul(ms_ps[:], ones64[:], mask_f[:], start=True, stop=True)
            emit_sub(7, part=0)
            emit_sq_mm(7, part=0)
            for b in range(BPC):
                nc.vector.tensor_reduce(
                    ms_sb[0:1, b : b + 1],
                    ms_ps[0:1, b * 64 : (b + 1) * 64],
                    mybir.AxisListType.X,
                    Op.add,
                )
            emit_sub(7, part=1)
            emit_sq_mm(7, part=1)
            emit_sub(7, part=2)
            emit_sq_mm(7, part=2)
            emit_dot(cs0_sb, 0, 0, HWH, 0)
            for pi, (col0, ncols) in enumerate(TAIL_PARTS):
                emit_dot(cs_ps[1], 1, col0, ncols, 1 + pi)
            nc.scalar.dma_start(out[:], ccol[:])
            nc.scalar.dma_start(out2[:], ms_sb[:])

    nc.compile()
    return nc


_NC_CACHE = {}


def _get_nc():
    if "nc" not in _NC_CACHE:
        _NC_CACHE["nc"] = build_nc()
    return _NC_CACHE["nc"]


def make_in_maps(map_t2, map_s2, priors, mimic_label):
    in_maps = []
    sel = np.zeros((128, BPC, BPC), dtype=np.float32)
    for b in range(BPC):
        sel[:, b, b] = 1.0
    pr = np.asarray(priors, dtype=np.float32)
    # priors_r[l, j*64 + c] = priors[l*64 + c, j]
    priors_r = np.ascontiguousarray(
        pr.reshape(128, NCHUNK, 4).transpose(0, 2, 1).reshape(128, 4 * NCHUNK)
    )
    for ci in range(N_CORES):
        sl = slice(ci * BPC, (ci + 1) * BPC)
        lab = np.asarray(mimic_label[sl]).astype(np.int32)  # [BPC, P]
        # labels_r[l, c*BPC + b] = lab[b, l*64 + c]
        labels_r = np.ascontiguousarray(
            lab.reshape(BPC, 128, NCHUNK).transpose(1, 2, 0).reshape(128, NCHUNK * BPC)
        )
        in_maps.append(
            {
                "s": np.ascontiguousarray(map_s2[sl]).astype(np.float32),
                "t": np.ascontiguousarray(map_t2[sl]).astype(np.float32),
                "priors_r": priors_r,
                "sel": sel,
                "labels_r": labels_r,
            }
        )
    return in_maps


def finish_host(core_outs):
    """core_outs: list of ([BPC, 4], [1, BPC]) pairs -> loss (float32)."""
    contribs = np.empty(B, np.float64)
    msums = np.empty(B, np.float64)
    for ci in range(N_CORES):
        o, o2 = core_outs[ci]
        o = np.asarray(o, dtype=np.float64)
        o2 = np.asarray(o2, dtype=np.float64)
        for b in range(BPC):
            contribs[ci * BPC + b] = o[b, 0] + o[b, 1] + o[b, 2] + o[b, 3]
            msums[ci * BPC + b] = o2[0, b]
    loss = 0.0
    for i in range(B):
        if msums[i] == 0.0:
            loss = 0.0
        else:
            loss = loss + contribs[i] / msums[i] / C
    return np.float32(loss / B)


def kernel(map_t2, map_s2, priors, mimic_label):
    from concourse.bass_utils import run_bass_kernel_spmd

    nc = _get_nc()
    in_maps = make_in_maps(map_t2, map_s2, priors, mimic_label)
    res = run_bass_kernel_spmd(nc, in_maps, core_ids=list(range(N_CORES)))
    outs = [
        (res.results[ci]["out"], res.results[ci]["out2"]) for ci in range(N_CORES)
    ]
    return finish_host(outs)


# revision 21
# speedup vs baseline: 1.0887x; 1.0887x over previous
"""Trainium2 Bass kernel for nn_MimicLoss (masked-MSE mimic loss), v9.

Data-parallel over batch: 8 NeuronCores x 4 samples. Per core:
  1. rasterize per-sample union-of-positive-boxes masks: priors arrive
     host-transposed so cx/cy/w/h are contiguous [128, 64] rows;
     coverage built in 6 batched [128, 4096] bf16 DVE ops, xb4
     (cov_x * pos) batched 16 chunks per DVE op, then one bf16 matmul
     per 128-prior chunk accumulates
       hit[h, (b,w)] += sum_p cov_y[p,h] * (cov_x[p,w] * pos[p,b])
     mask = hit > 0; mask also transposed to per-sample row layout
     mask_rows[b, (h w)] via 4 small SBUF->SBUF DMAs.
  2. stream units u = (pixel-half h, sample b): tile [128, cc, j] f32
     holds s[b, :, h*2048:(h+1)*2048]; s and t interleave on the sync
     HWDGE queue (one queue sustains ~385 GB/s). d = s - t in place in
     the s tile; subtracts are deadline-scheduled: DVE interleaves them
     between raster ops so each unit's subtract lands before its t/s
     buffer must recycle, with units 2 and 4 offloaded to GpSimd (kept
     few: GpSimd tensor ops degrade concurrent DVE throughput). ACT
     squares to f32r; PE accumulates per-sample column sums into ONE
     psum tile [4, 2048] per pixel-half using per-sample indicator
     stationaries sel_b [128, 4] (only column b set), so sample b's
     colsum accumulates in psum row b across the phase.
  3. per pixel-half: DVE multiplies the psum rows directly with
     mask_rows into scr, one reduce per region -> ccol columns; msum
     via one reduce over mask_rows. Host combines the [4, 4] outputs:
     contrib = cols 0..2, msum = col 3; applies /(msum*C) and the
     empty-mask-resets-loss scan, /B.

Self-contained: shapes hardcoded for map_t2/map_s2 [32,256,64,64] f32,
priors [8192,4] f32, mimic_label [32,8192] int32.
"""
import sys

sys.path.insert(0, "/opt/trn_rl_repo")

import numpy as np

import concourse.bacc as bacc
import concourse.tile as tile
from concourse import mybir
from concourse.alu_op_type import AluOpType as Op

F32 = mybir.dt.float32
F32R = mybir.dt.float32r
I32 = mybir.dt.int32
BF16 = mybir.dt.bfloat16
AF = mybir.ActivationFunctionType

B, C, H, W = 32, 256, 64, 64
P = 8192
N_CORES = 8
BPC = B // N_CORES          # samples per core
HW = H * W                  # 4096
HWH = HW // 2               # 2048 pixels per half
HWQ = HWH // 2              # 1024 pixels per tail part
NCHUNK = 64                 # prior chunks; prior p = lane*64 + c
CC = C // 128               # channel chunks
GCH = 16                    # chunks per xb4 batch group
NGR = NCHUNK // GCH         # 4 raster groups
NU = 2 * BPC                # stream units (pixel-half, sample)
GP_SUB_UNITS = ()           # GpSimd tensor ops degrade concurrent DVE ops


def build_nc():
    nc = bacc.Bacc("TRN2", debug=False)

    s = nc.dram_tensor("s", [BPC, C, H, W], F32, kind="ExternalInput")
    t = nc.dram_tensor("t", [BPC, C, H, W], F32, kind="ExternalInput")
    # host-transposed, contiguous per partition:
    # priors_r[l, j*64 + c] = priors[l*64 + c, j]
    priors_r = nc.dram_tensor("priors_r", [128, 4 * NCHUNK], F32, kind="ExternalInput")
    # labels_r[l, c*BPC + b] = mimic_label[b, l*64 + c] (host-permuted)
    labels_r = nc.dram_tensor("labels_r", [128, NCHUNK * BPC], I32, kind="ExternalInput")
    # sel[l, b, :] = indicator row: only column b set (host-provided f32r)
    sel_in = nc.dram_tensor("sel", [128, BPC, BPC], F32R, kind="ExternalInput")
    out = nc.dram_tensor("out", [BPC, 4], F32, kind="ExternalOutput")
    out2 = nc.dram_tensor("out2", [1, BPC], F32, kind="ExternalOutput")

    # unit (b, h) tile layout: [p, cc, j] = x[b, cc*128 + p, h*2048 + j]
    s_r = s[:].rearrange("b (cc p) (h r) w -> b h p cc (r w)", cc=CC, h=2)
    t_r = t[:].rearrange("b (cc p) (h r) w -> b h p cc (r w)", cc=CC, h=2)

    with tile.TileContext(nc) as tc:
        with (
            tc.tile_pool(name="const", bufs=1) as constp,
            tc.tile_pool(name="small", bufs=1) as small,
            tc.tile_pool(name="rast", bufs=1) as rastp,
            tc.tile_pool(name="xb4p", bufs=1) as xb4p,
            tc.tile_pool(name="stream_s", bufs=3) as pool_s,
            tc.tile_pool(name="stream_t", bufs=3) as pool_t,
            tc.tile_pool(name="d2p", bufs=2) as d2p,
            tc.tile_pool(name="ps_hit", bufs=1, space="PSUM") as ps_hit,
            tc.tile_pool(name="ps_cs", bufs=1, space="PSUM") as ps_cs,
            tc.tile_pool(name="ps_ms", bufs=1, space="PSUM") as ps_ms,
        ):
            # ---- constants & small inputs ----
            # sel3d[:, b, :] = stationary for sample b (only column b set)
            sel_r = constp.tile([128, BPC, BPC], F32R)
            nc.scalar.dma_start(sel_r[:], sel_in[:])

            priors_sb = small.tile([128, 4, NCHUNK], F32)
            nc.scalar.dma_start(
                priors_sb[:].rearrange("l j c -> l (j c)"), priors_r[:]
            )
            labels_sb = small.tile([128, NCHUNK, BPC], I32)
            nc.scalar.dma_start(
                labels_sb[:], labels_r[:].rearrange("l (c b) -> l c b", b=BPC)
            )

            # ---- stream tiles + DMA emission (sync HWDGE queue) ----
            # unit u: h = u // BPC, b = u % BPC. Last unit split into
            # column-halves (parts) for a progressive tail drain.
            def unit_bh(u):
                h, b = divmod(u, BPC)
                return b, h

            # s on the sync HWDGE ring; t on the scalar HWDGE ring.
            # t3.. are issued later (interleaved after squares) so the
            # t-buffer rotation waits never block ACT's sequencer.
            TAIL_PARTS = ((0, 1024), (1024, 512), (1536, 512))
            unit_parts = [None] * NU  # list of (s_tile, t_tile, col0, ncols)
            for u in range(NU):
                b, h = unit_bh(u)
                s_t = pool_s.tile([128, CC, HWH], F32, tag="s", name=f"s_{u}")
                t_t = pool_t.tile([128, CC, HWH], F32, tag="t", name=f"t_{u}")
                if u < NU - 1:
                    nc.sync.dma_start(s_t[:], s_r[b, h])
                    unit_parts[u] = [(s_t, t_t, 0, HWH)]
                else:
                    parts = []
                    for col0, ncols in TAIL_PARTS:
                        cols = slice(col0, col0 + ncols)
                        nc.sync.dma_start(s_t[:, :, cols], s_r[b, h][:, :, cols])
                        parts.append((s_t, t_t, col0, ncols))
                    unit_parts[u] = parts

            def emit_t_load(u):
                b, h = unit_bh(u)
                for s_t, t_t, col0, ncols in unit_parts[u]:
                    cols = slice(col0, col0 + ncols)
                    nc.scalar.dma_start(t_t[:, :, cols], t_r[b, h][:, :, cols])

            for u in range(3):
                emit_t_load(u)

            # ---- raster tiles ----
            iota_rep = rastp.tile([128, NCHUNK, 64], BF16)
            covx_all = rastp.tile([128, NCHUNK, 64], BF16)
            covy_all = rastp.tile([128, NCHUNK, 64], BF16)
            tmpB = rastp.tile([128, NCHUNK, 64], BF16)
            pos_f = small.tile([128, NCHUNK * BPC], F32)
            pos_bf = small.tile([128, NCHUNK, BPC], BF16)
            hw_half = small.tile([128, NCHUNK], F32)
            hh_half = small.tile([128, NCHUNK], F32)
            xm1 = small.tile([128, NCHUNK], F32)
            xx1 = small.tile([128, NCHUNK], F32)
            ym1 = small.tile([128, NCHUNK], F32)
            yy1 = small.tile([128, NCHUNK], F32)

            hit = ps_hit.tile([64, BPC * 64], F32)
            mask_f = small.tile([64, BPC * 64], F32)
            mask_rows = small.tile([BPC, HW], F32)
            cs0_sb = small.tile([BPC, HWH], F32)
            ccol = small.tile([BPC, 4], F32)
            ones64 = constp.tile([64, 1], F32)
            ms_ps = ps_ms.tile([1, BPC * 64], F32)
            ms_sb = small.tile([1, BPC], F32)

            def bcast(bnd):
                return bnd[:].rearrange("p (c o) -> p c o", o=1).broadcast_to(
                    [128, NCHUNK, 64]
                )

            # ---- gpsimd program start: iota ----
            nc.gpsimd.iota(
                iota_rep[:],
                pattern=[[0, NCHUNK], [1, 64]],
                base=0,
                channel_multiplier=0,
                allow_small_or_imprecise_dtypes=True,
            )

            # ---- engine-op emitters (called in deadline order below) ----
            def emit_prep():
                nc.vector.memset(ones64[:], 1.0)
                nc.vector.tensor_copy(
                    pos_f[:], labels_sb[:].rearrange("p c b -> p (c b)")
                )
                nc.vector.tensor_single_scalar(pos_f[:], pos_f[:], 0.0, Op.is_gt)
                nc.vector.tensor_copy(
                    pos_bf[:].rearrange("p c b -> p (c b)"), pos_f[:]
                )
                cx = priors_sb[:, 0, :]
                cy = priors_sb[:, 1, :]
                bw = priors_sb[:, 2, :]
                bh = priors_sb[:, 3, :]
                nc.vector.tensor_single_scalar(hw_half[:], bw, 0.5, Op.mult)
                nc.vector.tensor_single_scalar(hh_half[:], bh, 0.5, Op.mult)
                nc.vector.tensor_tensor(xm1[:], cx, hw_half[:], Op.subtract)
                nc.vector.tensor_scalar(xm1[:], xm1[:], 64.0, -1.0, Op.mult, Op.add)
                nc.vector.tensor_tensor(xx1[:], cx, hw_half[:], Op.add)
                nc.vector.tensor_scalar(xx1[:], xx1[:], 64.0, -1.0, Op.mult, Op.add)
                nc.vector.tensor_tensor(ym1[:], cy, hh_half[:], Op.subtract)
                nc.vector.tensor_scalar(ym1[:], ym1[:], 64.0, -1.0, Op.mult, Op.add)
                nc.vector.tensor_tensor(yy1[:], cy, hh_half[:], Op.add)
                nc.vector.tensor_scalar(yy1[:], yy1[:], 64.0, -1.0, Op.mult, Op.add)

            cs_ps = [None, None]

            def emit_sub(u, part=None):
                b, h = unit_bh(u)
                parts = unit_parts[u] if part is None else [unit_parts[u][part]]
                for s_t, t_t, col0, ncols in parts:
                    cols = slice(col0, col0 + ncols)
                    eng = nc.gpsimd if u in GP_SUB_UNITS else nc.vector
                    eng.tensor_tensor(
                        s_t[:, :, cols], s_t[:, :, cols], t_t[:, :, cols],
                        Op.subtract,
                    )

            def emit_sq_mm(u, part=None):
                b, h = unit_bh(u)
                if cs_ps[h] is None or (b == 0 and part in (None, 0)):
                    if cs_ps[h] is None:
                        cs_ps[h] = ps_cs.tile(
                            [BPC, HWH], F32, tag="cs", name=f"cs{h}"
                        )
                first = b == 0
                last = b == BPC - 1
                parts = unit_parts[u] if part is None else [unit_parts[u][part]]
                for s_t, t_t, col0, ncols in parts:
                    cols = slice(col0, col0 + ncols)
                    d2 = d2p.tile([128, CC, HWH], F32R, tag="d2", name="d2")
                    nc.scalar.activation(d2[:, :, cols], s_t[:, :, cols], AF.Square)
                    if part in (None, 0) and u + 3 < NU:
                        emit_t_load(u + 3)
                    nq = ncols // 512
                    for cc in range(CC):
                        for q in range(nq):
                            c0 = col0 + q * 512
                            nc.tensor.matmul(
                                cs_ps[h][0:BPC, c0 : c0 + 512],
                                sel_r[:, b, :],
                                d2[:, cc, c0 : c0 + 512],
                                start=(first and cc == 0),
                                stop=(last and cc == CC - 1),
                            )

            def emit_xb4_mms(g):
                xb4 = xb4p.tile([128, GCH, BPC, 64], BF16, tag="xb4", name="xb4")
                covx_v = (
                    covx_all[:, g * GCH : (g + 1) * GCH, :]
                    .rearrange("p c (o w) -> p c o w", o=1)
                    .broadcast_to([128, GCH, BPC, 64])
                )
                pos_v = (
                    pos_bf[:, g * GCH : (g + 1) * GCH, :]
                    .rearrange("p c (b o) -> p c b o", o=1)
                    .broadcast_to([128, GCH, BPC, 64])
                )
                nc.vector.tensor_tensor(xb4[:], covx_v, pos_v, Op.mult)
                for k in range(GCH):
                    c = g * GCH + k
                    nc.tensor.matmul(
                        hit[:],
                        covy_all[:, c, :],
                        xb4[:, k].rearrange("p b w -> p (b w)"),
                        start=(c == 0),
                        stop=(c == NCHUNK - 1),
                    )

            def emit_mask_rows():
                # 4 SBUF->SBUF DMAs: [64, 64] block -> one [1, 4096] row
                for b in range(BPC):
                    nc.gpsimd.dma_start(
                        mask_rows[b : b + 1, :].rearrange("o (p j) -> o p j", j=64),
                        mask_f[:, b * 64 : (b + 1) * 64],
                    )

            def emit_dot(src_ap, h, col0, ncols, ccol_idx):
                # in-place: mask_rows *= cs rows, then reduce the region
                cols = slice(col0, col0 + ncols)
                mcols = slice(h * HWH + col0, h * HWH + col0 + ncols)
                nc.vector.tensor_tensor(
                    mask_rows[:, mcols], src_ap[:, cols], mask_rows[:, mcols],
                    Op.mult,
                )
                nc.vector.tensor_reduce(
                    ccol[:, ccol_idx : ccol_idx + 1],
                    mask_rows[:, mcols],
                    mybir.AxisListType.X,
                    Op.add,
                )

            # ---- explicit deadline-ordered schedule ----
            # DVE: prep -> sub0 -> 2 coverage ops -> sub1 -> 4 coverage
            # ops -> xb4 g0 -> sub3 -> xb4 g1,g2 -> sub5 -> xb4 g3 ->
            # mask -> sub6 -> msum -> dot_h0 -> sub7a,b -> dots h1.
            # GpSimd: iota -> sub2 -> sub4 -> mask_rows.
            emit_prep()
            emit_sub(0)
            emit_sq_mm(0)
            nc.vector.tensor_tensor(covy_all[:], bcast(ym1), iota_rep[:], Op.is_lt)
            nc.vector.tensor_tensor(covx_all[:], bcast(xm1), iota_rep[:], Op.is_lt)
            emit_sub(1)
            emit_sq_mm(1)
            nc.vector.tensor_tensor(tmpB[:], bcast(xx1), iota_rep[:], Op.is_ge)
            nc.vector.tensor_tensor(covx_all[:], covx_all[:], tmpB[:], Op.mult)
            nc.vector.tensor_tensor(tmpB[:], bcast(yy1), iota_rep[:], Op.is_ge)
            nc.vector.tensor_tensor(covy_all[:], covy_all[:], tmpB[:], Op.mult)
            emit_sub(2)       # GpSimd
            emit_sq_mm(2)
            emit_xb4_mms(0)
            emit_sub(3)
            emit_sq_mm(3)
            # h0 psum closed: one cheap ACT copy frees the psum buffer
            # so PE's h1 start-matmuls don't wait on the late h0 dot
            nc.scalar.copy(cs0_sb[:], cs_ps[0][:])
            emit_xb4_mms(1)
            emit_sub(4)       # GpSimd
            emit_sq_mm(4)
            emit_xb4_mms(2)
            emit_sub(5)
            emit_sq_mm(5)
            emit_xb4_mms(3)
            nc.vector.tensor_single_scalar(mask_f[:], hit[:], 0.0, Op.is_gt)
            emit_mask_rows()  # GpSimd
            # msum: PE column-sum of mask_f, then 4 tiny per-sample reduces
            nc.tensor.matmul(ms_ps[:], ones64[:], mask_f[:], start=True, stop=True)
            for b in range(BPC):
                nc.vector.tensor_reduce(
                    ms_sb[0:1, b : b + 1],
                    ms_ps[0:1, b * 64 : (b + 1) * 64],
                    mybir.AxisListType.X,
                    Op.add,
                )
            emit_dot(cs0_sb, 0, 0, HWH, 0)
            emit_sub(6)
            emit_sq_mm(6)
            for pi in range(3):
                emit_sub(7, part=pi)
                emit_sq_mm(7, part=pi)
            for pi, (col0, ncols) in enumerate(TAIL_PARTS):
                emit_dot(cs_ps[1], 1, col0, ncols, 1 + pi)
            nc.sync.dma_start(out[:], ccol[:])
            nc.sync.dma_start(out2[:], ms_sb[:])

    nc.compile()
    return nc


_NC_CACHE = {}


def _get_nc():
    if "nc" not in _NC_CACHE:
        _NC_CACHE["nc"] = build_nc()
    return _NC_CACHE["nc"]


def make_in_maps(map_t2, map_s2, priors, mimic_label):
    in_maps = []
    sel = np.zeros((128, BPC, BPC), dtype=np.float32)
    for b in range(BPC):
        sel[:, b, b] = 1.0
    pr = np.asarray(priors, dtype=np.float32)
    # priors_r[l, j*64 + c] = priors[l*64 + c, j]
    priors_r = np.ascontiguousarray(
        pr.reshape(128, NCHUNK, 4).transpose(0, 2, 1).reshape(128, 4 * NCHUNK)
    )
    for ci in range(N_CORES):
        sl = slice(ci * BPC, (ci + 1) * BPC)
        lab = np.asarray(mimic_label[sl]).astype(np.int32)  # [BPC, P]
        # labels_r[l, c*BPC + b] = lab[b, l*64 + c]
        labels_r = np.ascontiguousarray(
            lab.reshape(BPC, 128, NCHUNK).transpose(1, 2, 0).reshape(128, NCHUNK * BPC)
        )
        in_maps.append(
            {
                "s": np.ascontiguousarray(map_s2[sl]).astype(np.float32),
                "t": np.ascontiguousarray(map_t2[sl]).astype(np.float32),
                "priors_r": priors_r,
                "sel": sel,
                "labels_r": labels_r,
            }
        )
    return in_maps


def finish_host(core_outs):
    """core_outs: list of ([BPC, 4], [1, BPC]) pairs -> loss (float32)."""
    contribs = np.empty(B, np.float64)
    msums = np.empty(B, np.float64)
    for ci in range(N_CORES):
        o, o2 = core_outs[ci]
        o = np.asarray(o, dtype=np.float64)
        o2 = np.asarray(o2, dtype=np.float64)
        for b in range(BPC):
            contribs[ci * BPC + b] = o[b, 0] + o[b, 1] + o[b, 2] + o[b, 3]
            msums[ci * BPC + b] = o2[0, b]
    loss = 0.0
    for i in range(B):
        if msums[i] == 0.0:
            loss = 0.0
        else:
            loss = loss + contribs[i] / msums[i] / C
    return np.float32(loss / B)


def kernel(map_t2, map_s2, priors, mimic_label):
    from concourse.bass_utils import run_bass_kernel_spmd

    nc = _get_nc()
    in_maps = make_in_maps(map_t2, map_s2, priors, mimic_label)
    res = run_bass_kernel_spmd(nc, in_maps, core_ids=list(range(N_CORES)))
    outs = [
        (res.results[ci]["out"], res.results[ci]["out2"]) for ci in range(N_CORES)
    ]
    return finish_host(outs)
